# revision 41
# baseline (speedup 1.0000x reference)
"""Trainium2 Bass kernel for BoT-style attention (nn_Attention_20968030339767).

Data-parallel over batch: 16 batches -> 2 per NeuronCore, 8 cores, no
collectives.  All BN folding / bias-table exponentiation happens on host;
the device runs projections + attention + hardswish + output projection.

Math (per batch b):
  q = BN(Wq x), k = BN(Wk x), v = BN(Wv x)          (1x1 conv == channel matmul)
  logits = SCALE*(q.k) + emb[pos]/SCALE
  attn   = softmax(logits); out = attn @ v
  hs     = hardswish(out); y = BN(W_out hs + b_out)

Device-side formulation (per core):
  qT = Wq'' x   [512, 1024]  (Wq'' = SCALE*diag(sq)*Wq, + bias bq'' at evac)
  kT = Wk'  x   [512, 1024]
  vT = x^T Wv'^T + bv'       [1024, 520]  (65-wide per-head blocks: 64 v + ones)
  per (h, b): ST_j = kT_h[:,jtile]^T qT_h       [128 j, 1024 i]  (logits^T)
              P_j  = exp(ST_j) * EB_h[jtile]    (EB = exp(emb[pos]/SCALE), bf16)
              UT  += vT_aug_j^T @ P_j           [65, 1024]  (row 64 = sumexp)
              since vT carries bv': UT[0:64] = U_raw + bv'*sumexp
              recip = 1/UT[64]; u = UT[0:64] * bcast(recip)  (= U/sum + bv')
              hs6 = u * clip(u+3, 0, 6)        (the /6 is folded into Wo'')
  y = Wo'' hs6 + bo''   (Wo'' = diag(so)*W_out/6)
"""

import os
import sys
import functools

import numpy as np

sys.path.insert(0, "/opt/trn_rl_repo")

import ml_dtypes  # noqa: E402

from concourse import bacc, mybir  # noqa: E402
import concourse.bass as bass  # noqa: E402
import concourse.tile as tile  # noqa: E402
from concourse.bass_utils import run_bass_kernel_spmd  # noqa: E402

BF16 = mybir.dt.bfloat16
F32 = mybir.dt.float32
Af = mybir.ActivationFunctionType
Op = mybir.AluOpType

B, DIM, H, DK, N = 16, 256, 8, 64, 1024
INK = H * DK  # 512
SCALE = DK ** -0.5
EPS = 1e-5
NCORES = 8
BPC = B // NCORES  # batches per core


def build_body(nc, tc, d):
    """Emit the whole per-core program inside a TileContext."""
    ts = bass.ts

    _n = [0]
    pool = d["_pool"]
    psum_pool = d["_psum_pool"]

    def T(shape, dtype, **kw):
        if "name" not in kw:
            kw["name"] = f"{kw.get('tag', 't')}_{_n[0]}"
            _n[0] += 1
        space = kw.pop("space", None)
        if space == "PSUM":
            p = psum_pool
        elif space == "DRAM":
            p = d["_dram_pool"]
        else:
            p = pool
        return p.tile(shape, dtype, **kw)

    # ---- persistent SBUF tensors (weights packed into one DMA) -----------
    wpk = T([128, 4096], BF16, tag="wpk", bufs=1)
    bpk = T([128, 10], F32, tag="bpk", bufs=1)
    bv_bc = T([128, 512], BF16, tag="bv_bc", bufs=1)
    nc.sync.dma_start(wpk[:], d["wpack"][:])
    nc.sync.dma_start(bpk[:], d["bpack"][:])
    # bv broadcast to all 128 partitions (token rows) via step-0 DMA
    nc.sync.dma_start(bv_bc[:], d["bv"].broadcast_to([128, 512]))
    wq_s = wpk[:].rearrange("p (w k o) -> p w k o", w=4, k=2)[:, 0]
    wk_s = wpk[:].rearrange("p (w k o) -> p w k o", w=4, k=2)[:, 1]
    wv_s = wpk[:].rearrange("p (w k o) -> p w k o", w=4, k=2)[:, 2]
    wo_s = wpk[:, 3072:4096].rearrange("p (k o) -> p k o", k=4)
    bq_s = bpk[:, 0:4]
    bk_s = bpk[:, 4:8]
    bo_s = bpk[:, 8:10]

    x_s = []
    for b in range(BPC):
        xt = T([128, 2, N], BF16, tag="x", bufs=BPC, name=f"x{b}")
        nc.sync.dma_start(xt[:], d["x"][b].rearrange("k p n -> p k n"))
        x_s.append(xt)

    # ---- phase 1: projections -------------------------------------------
    q_s = [[None] * 4 for _ in range(BPC)]
    k_s = [[None] * 4 for _ in range(BPC)]
    v_s = [[None] * 8 for _ in range(BPC)]
    for b in range(BPC):
        for m in range(4):
            qp = T([128, N], F32, space="PSUM", tag="stm", bufs=3, name=f"qp{b}{m}")
            for ic in range(2):
                for kc in range(2):
                    nc.tensor.matmul(
                        qp[:, ts(ic, 512)],
                        wq_s[:, kc, ts(m, 128)],
                        x_s[b][:, kc, ts(ic, 512)],
                        start=(kc == 0),
                        stop=(kc == 1),
                    )
            qt = T([128, N], BF16, tag="q", bufs=4 * BPC, name=f"q{b}{m}")
            nc.scalar.activation(qt[:], qp[:], Af.Identity, bias=bq_s[:, m : m + 1])
            q_s[b][m] = qt

            kp = T([128, N], F32, space="PSUM", tag="stm", bufs=3, name=f"kp{b}{m}")
            for ic in range(2):
                for kc in range(2):
                    nc.tensor.matmul(
                        kp[:, ts(ic, 512)],
                        wk_s[:, kc, ts(m, 128)],
                        x_s[b][:, kc, ts(ic, 512)],
                        start=(kc == 0),
                        stop=(kc == 1),
                    )
            kt = T([128, N], BF16, tag="k", bufs=4 * BPC, name=f"k{b}{m}")
            nc.vector.tensor_scalar(kt[:], kp[:], bk_s[:, m : m + 1], None, Op.add)
            k_s[b][m] = kt

        for t in range(8):
            vp = T([128, 512], F32, space="PSUM", tag="stm", bufs=3, name=f"vp{b}{t}")
            for kc in range(2):
                nc.tensor.matmul(
                    vp[:],
                    x_s[b][:, kc, ts(t, 128)],
                    wv_s[:, kc, :],
                    start=(kc == 0),
                    stop=(kc == 1),
                )
            # evac + bv'' add; bias rides into UT as bv*sumexp (softmax rows sum to 1)
            vt = T([128, 8, 65], BF16, tag="v", bufs=8 * BPC, name=f"v{b}{t}")
            nc.vector.memset(vt[:, :, 64:65], 1.0)
            nc.vector.tensor_add(
                vt[:, :, 0:64],
                vp[:].rearrange("p (h e) -> p h e", e=64),
                bv_bc[:].rearrange("p (h e) -> p h e", e=64),
            )
            v_s[b][t] = vt

    # ---- phase 2: attention ---------------------------------------------
    hs_s = [[None] * 4 for _ in range(BPC)]
    for b in range(BPC):
        for oc in range(4):
            hs_s[b][oc] = T([128, N], BF16, tag="hs", bufs=4 * BPC, name=f"hs{b}{oc}")

    def yproj(b):
        for m in range(2):
            for t2 in range(2):
                yp = T([128, 512], F32, space="PSUM", tag="stm", bufs=3, name=f"yp{b}{m}{t2}")
                for kc in range(4):
                    nc.tensor.matmul(
                        yp[:],
                        wo_s[:, kc, ts(m, 128)],
                        hs_s[b][kc][:, ts(t2, 512)],
                        start=(kc == 0),
                        stop=(kc == 3),
                    )
                ys = T([128, 512], F32, tag="y", bufs=3, name=f"ys{b}{m}{t2}")
                nc.scalar.activation(ys[:], yp[:], Af.Identity, bias=bo_s[:, m : m + 1])
                nc.sync.dma_start(d["y"][b, m, :, ts(t2, 512)], ys[:])

    for h in range(H):
        eb = T([128, 8, N], BF16, tag="eb", bufs=2, name=f"eb{h}")
        nc.sync.dma_start(eb[:], d["eb"][h].rearrange("j p n -> p j n"))
        m2, r0 = h // 2, (h % 2) * 64
        for b in range(BPC):
            ut = T([65, N], F32, space="PSUM", tag="ut", bufs=1, name=f"ut{h}{b}")
            for j in range(8):
                st = T([128, N], F32, space="PSUM", tag="stm", bufs=3, name=f"st{h}{b}{j}")
                kT = k_s[b][m2][r0 : r0 + 64, ts(j, 128)]
                for ic in range(2):
                    nc.tensor.matmul(
                        st[:, ts(ic, 512)],
                        kT,
                        q_s[b][m2][r0 : r0 + 64, ts(ic, 512)],
                        start=True,
                        stop=True,
                    )
                e = T([128, N], BF16, tag="e", bufs=4, name=f"e{h}{b}{j}")
                nc.scalar.activation(e[:], st[:], Af.Exp)
                p = T([128, N], BF16, tag="p", bufs=4, name=f"p{h}{b}{j}")
                nc.vector.tensor_mul(p[:], e[:], eb[:, j, :])
                for ic in range(2):
                    nc.tensor.matmul(
                        ut[:, ts(ic, 512)],
                        v_s[b][j][:, h, :],
                        p[:, ts(ic, 512)],
                        start=(j == 0),
                        stop=(j == 7),
                    )
            # softmax denominators: UT row 64 = sum_j exp.
            # recip on [1,1024] is lane-starved (~6.5us) -> DMA-transpose to
            # [128,8], recip there, cast bf16, DMA-broadcast to [64,1024].
            sums = T([1, N], F32, tag="sums", bufs=2, name=f"sm{h}{b}")
            nc.scalar.copy(sums[:], ut[64:65, :])
            uraw = T([64, N], BF16, tag="uraw", bufs=2, name=f"ur{h}{b}")
            nc.vector.tensor_copy(uraw[:], ut[0:64, :])
            sums_dr = T([1, N], F32, tag="sums_dr", bufs=2, space="DRAM", name=f"smd{h}{b}")
            nc.sync.dma_start(sums_dr[:], sums[:])
            sumsT = T([128, 8], F32, tag="sumsT", bufs=2, name=f"smT{h}{b}")
            nc.sync.dma_start(
                sumsT[:], sums_dr[:].rearrange("o (p e) -> (o p) e", p=128)
            )
            recT = T([128, 8], F32, tag="recT", bufs=2, name=f"rcT{h}{b}")
            nc.vector.reciprocal(recT[:], sumsT[:])
            recTb = T([128, 8], BF16, tag="recTb", bufs=2, name=f"rcTb{h}{b}")
            nc.vector.tensor_copy(recTb[:], recT[:])
            rc_dram = T([1, N], BF16, tag="rc_dram", bufs=2, space="DRAM", name=f"rcd{h}{b}")
            nc.sync.dma_start(
                rc_dram[:].rearrange("o (p e) -> (o p) e", p=128), recTb[:]
            )
            bcast = T([64, N], BF16, tag="bcast", bufs=2, name=f"bc{h}{b}")
            nc.sync.dma_start(bcast[:], rc_dram[:].broadcast_to([64, N]))
            u = T([64, N], BF16, tag="u", bufs=2, name=f"u{h}{b}")
            nc.vector.tensor_mul(u[:], uraw[:], bcast[:])
            # hs6 = clip(u+3, 0, 6) * u  (the /6 lives in Wo'');
            # clip via two 4x-mode tensor_scalar ops, then a 2x tensor_tensor
            t1 = T([64, N], BF16, tag="t1", bufs=2, name=f"t1{h}{b}")
            nc.vector.tensor_scalar(t1[:], u[:], 3.0, 0.0, Op.add, Op.max)
            t2 = T([64, N], BF16, tag="t2", bufs=2, name=f"t2{h}{b}")
            nc.vector.tensor_scalar(t2[:], t1[:], 6.0, None, Op.min)
            nc.vector.tensor_mul(hs_s[b][m2][r0 : r0 + 64, :], t2[:], u[:])

    # ---- phase 3: output projection -------------------------------------
    for b in range(BPC):
        yproj(b)




@functools.cache
def build_nc():
    nc = bacc.Bacc(
        "TRN2",
        target_bir_lowering=False,
        debug=False,
        enable_asserts=False,
        num_devices=NCORES,
    )
    d = {
        "x": nc.dram_tensor("x", [BPC, 2, 128, N], BF16, kind="ExternalInput"),
        "wpack": nc.dram_tensor("wpack", [128, 4096], BF16, kind="ExternalInput"),
        "bpack": nc.dram_tensor("bpack", [128, 10], F32, kind="ExternalInput"),
        "bv": nc.dram_tensor("bv", [1, 512], BF16, kind="ExternalInput"),
        "eb": nc.dram_tensor("eb", [H, 8, 128, N], BF16, kind="ExternalInput"),
        "y": nc.dram_tensor("y", [BPC, 2, 128, N], F32, kind="ExternalOutput"),
    }
    d = {k: (v.ap() if hasattr(v, "ap") else v) for k, v in d.items()}
    with tile.TileContext(nc) as tc:
        with (
            tc.tile_pool(name="main", bufs=1) as pool,
            tc.tile_pool(name="psum", bufs=2, space="PSUM") as psum_pool,
            tc.tile_pool(name="dram", bufs=2, space="DRAM") as dram_pool,
        ):
            d["_pool"] = pool
            d["_psum_pool"] = psum_pool
            d["_dram_pool"] = dram_pool
            build_body(nc, tc, d)
    nc.compile()
    return nc


def _prep_inputs(inputs):
    f = lambda k: np.asarray(inputs[k], np.float32)
    x = f("x")
    sq = f("gq") / np.sqrt(f("vq") + EPS)
    sk = f("gk") / np.sqrt(f("vk") + EPS)
    sv = f("gv") / np.sqrt(f("vv") + EPS)
    so = f("go") / np.sqrt(f("vo") + EPS)
    Wq = f("wq") * sq[:, None] * SCALE
    bq = (f("bq") - f("mq") * sq) * SCALE
    Wk = f("wk") * sk[:, None]
    bk = f("bk") - f("mk") * sk
    Wv = f("wv") * sv[:, None]
    bv = f("bv") - f("mv") * sv
    Wo = f("w_out") * so[:, None] / 6.0
    bo = so * f("b_out") + f("bo") - f("mo") * so

    emb = f("emb")
    pos = np.asarray(inputs["pos_indices"], np.int64)
    EB = np.exp(emb[pos].transpose(2, 0, 1) / SCALE)  # [H, N, N]

    bf = ml_dtypes.bfloat16
    wpack = np.concatenate(
        [
            Wq.T.reshape(2, 128, 512).transpose(1, 0, 2).reshape(128, 1024),
            Wk.T.reshape(2, 128, 512).transpose(1, 0, 2).reshape(128, 1024),
            Wv.T.reshape(2, 128, 512).transpose(1, 0, 2).reshape(128, 1024),
            Wo.T.reshape(4, 128, 256).transpose(1, 0, 2).reshape(128, 1024),
        ],
        axis=1,
    )
    bpack = np.concatenate(
        [bq.reshape(4, 128).T, bk.reshape(4, 128).T, bo.reshape(2, 128).T], axis=1
    )
    shared = {
        "wpack": np.ascontiguousarray(wpack).astype(bf),
        "bpack": np.ascontiguousarray(bpack),
        "bv": bv.reshape(1, 512).astype(bf),
        "eb": np.ascontiguousarray(EB.reshape(H, 8, 128, N)).astype(bf),
    }
    x_dev = x.reshape(B, 2, 128, N).astype(bf)
    in_maps = [
        dict(shared, x=np.ascontiguousarray(x_dev[c * BPC : (c + 1) * BPC]))
        for c in range(NCORES)
    ]
    return in_maps


def kernel(**inputs):
    nc = build_nc()
    in_maps = _prep_inputs(inputs)
    res = run_bass_kernel_spmd(nc, in_maps, core_ids=list(range(NCORES)))
    y = np.concatenate([r["y"].reshape(BPC, DIM, 32, 32) for r in res.results], axis=0)
    return y.astype(np.float32)


def _install_ntff_hook():
    """The image's antenv lacks axon_hooks; synthesize it so trace=True works."""
    import types

    try:
        from antenv.axon_hooks import get_axon_ntff_profile_hook  # noqa: F401

        return
    except ImportError:
        pass
    import antenv
    from trn_agent_boot.trn_boot import _ntff_profile_via_ctypes

    mod = types.ModuleType("antenv.axon_hooks")
    mod._hook = _ntff_profile_via_ctypes("/opt/axon/libaxon_pjrt.so")
    mod.get_axon_ntff_profile_hook = lambda: mod._hook
    mod.set_axon_ntff_profile_hook = lambda h: setattr(mod, "_hook", h)
    sys.modules["antenv.axon_hooks"] = mod
    antenv.axon_hooks = mod

    # no artifact bucket in this container; neuter the upload
    import concourse.bass_utils as bu

    bu.upload_artifacts = lambda tmpdir: f"local:{tmpdir}"


def run_traced(inputs, tmpdir=None):
    """Like kernel() but with NTFF tracing; returns (y, BassKernelResults)."""
    _install_ntff_hook()
    nc = build_nc()
    in_maps = _prep_inputs(inputs)
    res = run_bass_kernel_spmd(
        nc, in_maps, core_ids=list(range(NCORES)), trace=True, tmpdir=tmpdir
    )
    y = np.concatenate([r["y"].reshape(BPC, DIM, 32, 32) for r in res.results], axis=0)
    return y.astype(np.float32), res


# revision 42
# speedup vs baseline: 1.1808x; 1.1808x over previous
"""Trainium2 Bass kernel for BoT-style attention (nn_Attention_20968030339767).

Data-parallel over batch: 16 batches -> 2 per NeuronCore, 8 cores, no
collectives.  All BN folding / bias-table exponentiation happens on host;
the device runs projections + attention + hardswish + output projection.

Math (per batch b):
  q = BN(Wq x), k = BN(Wk x), v = BN(Wv x)          (1x1 conv == channel matmul)
  logits = SCALE*(q.k) + emb[pos]/SCALE
  attn   = softmax(logits); out = attn @ v
  hs     = hardswish(out); y = BN(W_out hs + b_out)

Device-side formulation (per core):
  qT = Wq'' x   [512, 1024]  (Wq'' = SCALE*diag(sq)*Wq, + bias bq'' at evac)
  kT = Wk'  x   [512, 1024]
  vT = x^T Wv'^T + bv'       [1024, 520]  (65-wide per-head blocks: 64 v + ones)
  per (h, b): ST_j = kT_h[:,jtile]^T qT_h       [128 j, 1024 i]  (logits^T)
              P_j  = exp(ST_j) * EB_h[jtile]    (EB = exp(emb[pos]/SCALE), bf16)
              UT  += vT_aug_j^T @ P_j           [65, 1024]  (row 64 = sumexp)
              since vT carries bv': UT[0:64] = U_raw + bv'*sumexp
              recip = 1/UT[64]; u = UT[0:64] * bcast(recip)  (= U/sum + bv')
              hs6 = u * clip(u+3, 0, 6)        (the /6 is folded into Wo'')
  y = Wo'' hs6 + bo''   (Wo'' = diag(so)*W_out/6)
"""

import os
import sys
import functools

import numpy as np

sys.path.insert(0, "/opt/trn_rl_repo")

import ml_dtypes  # noqa: E402

from concourse import bacc, mybir  # noqa: E402
import concourse.bass as bass  # noqa: E402
import concourse.tile as tile  # noqa: E402
from concourse.bass_utils import run_bass_kernel_spmd  # noqa: E402

BF16 = mybir.dt.bfloat16
F32 = mybir.dt.float32
Af = mybir.ActivationFunctionType
Op = mybir.AluOpType

B, DIM, H, DK, N = 16, 256, 8, 64, 1024
INK = H * DK  # 512
SCALE = DK ** -0.5
EPS = 1e-5
NCORES = 8
BPC = B // NCORES  # batches per core


def build_body(nc, tc, d):
    """Emit the whole per-core program inside a TileContext."""
    ts = bass.ts

    _n = [0]
    pool = d["_pool"]
    psum_pool = d["_psum_pool"]

    def T(shape, dtype, **kw):
        if "name" not in kw:
            kw["name"] = f"{kw.get('tag', 't')}_{_n[0]}"
            _n[0] += 1
        space = kw.pop("space", None)
        if space == "PSUM":
            p = psum_pool
        elif space == "DRAM":
            p = d["_dram_pool"]
        else:
            p = pool
        return p.tile(shape, dtype, **kw)

    # ---- persistent SBUF tensors (weights packed into one DMA) -----------
    wpk = T([128, 4096], BF16, tag="wpk", bufs=1)
    bpk = T([128, 10], F32, tag="bpk", bufs=1)
    bv_bc = T([128, 512], BF16, tag="bv_bc", bufs=1)
    nc.sync.dma_start(wpk[:], d["wpack"][:])
    nc.sync.dma_start(bpk[:], d["bpack"][:])
    # bv broadcast to all 128 partitions (token rows) via step-0 DMA
    nc.sync.dma_start(bv_bc[:], d["bv"].broadcast_to([128, 512]))
    wq_s = wpk[:].rearrange("p (w k o) -> p w k o", w=4, k=2)[:, 0]
    wk_s = wpk[:].rearrange("p (w k o) -> p w k o", w=4, k=2)[:, 1]
    wv_s = wpk[:].rearrange("p (w k o) -> p w k o", w=4, k=2)[:, 2]
    wo_s = wpk[:, 3072:4096].rearrange("p (k o) -> p k o", k=4)
    bq_s = bpk[:, 0:4]
    bk_s = bpk[:, 4:8]
    bo_s = bpk[:, 8:10]

    x_s = []
    for b in range(BPC):
        xt = T([128, 2, N], BF16, tag="x", bufs=BPC, name=f"x{b}")
        nc.sync.dma_start(xt[:], d["x"][b].rearrange("k p n -> p k n"))
        x_s.append(xt)

    # ---- phase 1: projections -------------------------------------------
    q_s = [[None] * 4 for _ in range(BPC)]
    k_s = [[None] * 4 for _ in range(BPC)]
    v_s = [[None] * 8 for _ in range(BPC)]
    for b in range(BPC):
        for m in range(4):
            qp = T([128, N], F32, space="PSUM", tag="stm", bufs=3, name=f"qp{b}{m}")
            for ic in range(2):
                for kc in range(2):
                    nc.tensor.matmul(
                        qp[:, ts(ic, 512)],
                        wq_s[:, kc, ts(m, 128)],
                        x_s[b][:, kc, ts(ic, 512)],
                        start=(kc == 0),
                        stop=(kc == 1),
                    )
            qt = T([128, N], BF16, tag="q", bufs=4 * BPC, name=f"q{b}{m}")
            nc.vector.tensor_scalar(qt[:], qp[:], bq_s[:, m : m + 1], None, Op.add)
            q_s[b][m] = qt

            kp = T([128, N], F32, space="PSUM", tag="stm", bufs=3, name=f"kp{b}{m}")
            for ic in range(2):
                for kc in range(2):
                    nc.tensor.matmul(
                        kp[:, ts(ic, 512)],
                        wk_s[:, kc, ts(m, 128)],
                        x_s[b][:, kc, ts(ic, 512)],
                        start=(kc == 0),
                        stop=(kc == 1),
                    )
            kt = T([128, N], BF16, tag="k", bufs=4 * BPC, name=f"k{b}{m}")
            nc.vector.tensor_scalar(kt[:], kp[:], bk_s[:, m : m + 1], None, Op.add)
            k_s[b][m] = kt

        for t in range(8):
            vp = T([128, 512], F32, space="PSUM", tag="stm", bufs=3, name=f"vp{b}{t}")
            for kc in range(2):
                nc.tensor.matmul(
                    vp[:],
                    x_s[b][:, kc, ts(t, 128)],
                    wv_s[:, kc, :],
                    start=(kc == 0),
                    stop=(kc == 1),
                )
            # evac + bv'' add; bias rides into UT as bv*sumexp (softmax rows sum to 1)
            vt = T([128, 8, 65], BF16, tag="v", bufs=8 * BPC, name=f"v{b}{t}")
            nc.vector.memset(vt[:, :, 64:65], 1.0)
            nc.vector.tensor_add(
                vt[:, :, 0:64],
                vp[:].rearrange("p (h e) -> p h e", e=64),
                bv_bc[:].rearrange("p (h e) -> p h e", e=64),
            )
            v_s[b][t] = vt

    # ---- phase 2: attention ---------------------------------------------
    hs_s = [[None] * 4 for _ in range(BPC)]
    for b in range(BPC):
        for oc in range(4):
            hs_s[b][oc] = T([128, N], BF16, tag="hs", bufs=4 * BPC, name=f"hs{b}{oc}")

    def yproj(b):
        for m in range(2):
            for t2 in range(2):
                yp = T([128, 512], F32, space="PSUM", tag="stm", bufs=3, name=f"yp{b}{m}{t2}")
                for kc in range(4):
                    nc.tensor.matmul(
                        yp[:],
                        wo_s[:, kc, ts(m, 128)],
                        hs_s[b][kc][:, ts(t2, 512)],
                        start=(kc == 0),
                        stop=(kc == 3),
                    )
                ys = T([128, 512], F32, tag="y", bufs=3, name=f"ys{b}{m}{t2}")
                nc.scalar.activation(ys[:], yp[:], Af.Identity, bias=bo_s[:, m : m + 1])
                nc.sync.dma_start(d["y"][b, m, :, ts(t2, 512)], ys[:])

    for h in range(H):
        eb = T([128, 8, N], BF16, tag="eb", bufs=2, name=f"eb{h}")
        nc.sync.dma_start(eb[:], d["eb"][h].rearrange("j p n -> p j n"))
        m2, r0 = h // 2, (h % 2) * 64
        for b in range(BPC):
            ut = T([65, N], F32, space="PSUM", tag="ut", bufs=1, name=f"ut{h}{b}")
            for j in range(8):
                st = T([128, N], F32, space="PSUM", tag="stm", bufs=3, name=f"st{h}{b}{j}")
                kT = k_s[b][m2][r0 : r0 + 64, ts(j, 128)]
                for ic in range(2):
                    nc.tensor.matmul(
                        st[:, ts(ic, 512)],
                        kT,
                        q_s[b][m2][r0 : r0 + 64, ts(ic, 512)],
                        start=True,
                        stop=True,
                    )
                e = T([128, N], BF16, tag="e", bufs=4, name=f"e{h}{b}{j}")
                nc.scalar.activation(e[:], st[:], Af.Exp)
                p = T([128, N], BF16, tag="p", bufs=4, name=f"p{h}{b}{j}")
                nc.vector.tensor_mul(p[:], e[:], eb[:, j, :])
                for ic in range(2):
                    nc.tensor.matmul(
                        ut[:, ts(ic, 512)],
                        v_s[b][j][:, h, :],
                        p[:, ts(ic, 512)],
                        start=(j == 0),
                        stop=(j == 7),
                    )
            # softmax denominators: UT row 64 = sum_j exp.
            # recip on [1,1024] is lane-starved (~6.5us) -> DMA-transpose to
            # [128,8], recip there, cast bf16, DMA-broadcast to [64,1024].
            sums = T([1, N], F32, tag="sums", bufs=2, name=f"sm{h}{b}")
            nc.scalar.copy(sums[:], ut[64:65, :])
            uraw = T([64, N], BF16, tag="uraw", bufs=2, name=f"ur{h}{b}")
            nc.vector.tensor_copy(uraw[:], ut[0:64, :])
            sums_dr = T([1, N], F32, tag="sums_dr", bufs=2, space="DRAM", name=f"smd{h}{b}")
            nc.sync.dma_start(sums_dr[:], sums[:])
            sumsT = T([128, 8], F32, tag="sumsT", bufs=2, name=f"smT{h}{b}")
            nc.sync.dma_start(
                sumsT[:], sums_dr[:].rearrange("o (p e) -> (o p) e", p=128)
            )
            recT = T([128, 8], F32, tag="recT", bufs=2, name=f"rcT{h}{b}")
            nc.vector.reciprocal(recT[:], sumsT[:])
            recTb = T([128, 8], BF16, tag="recTb", bufs=2, name=f"rcTb{h}{b}")
            nc.vector.tensor_copy(recTb[:], recT[:])
            rc_dram = T([1, N], BF16, tag="rc_dram", bufs=2, space="DRAM", name=f"rcd{h}{b}")
            nc.sync.dma_start(
                rc_dram[:].rearrange("o (p e) -> (o p) e", p=128), recTb[:]
            )
            bcast = T([64, N], BF16, tag="bcast", bufs=2, name=f"bc{h}{b}")
            nc.sync.dma_start(bcast[:], rc_dram[:].broadcast_to([64, N]))
            u = T([64, N], BF16, tag="u", bufs=2, name=f"u{h}{b}")
            nc.vector.tensor_mul(u[:], uraw[:], bcast[:])
            # hs6 = clip(u+3, 0, 6) * u  (the /6 lives in Wo'');
            # clip via two 4x-mode tensor_scalar ops, then a 2x tensor_tensor
            t1 = T([64, N], BF16, tag="t1", bufs=2, name=f"t1{h}{b}")
            nc.vector.tensor_scalar(t1[:], u[:], 3.0, 0.0, Op.add, Op.max)
            t2 = T([64, N], BF16, tag="t2", bufs=2, name=f"t2{h}{b}")
            nc.vector.tensor_scalar(t2[:], t1[:], 6.0, None, Op.min)
            nc.vector.tensor_mul(hs_s[b][m2][r0 : r0 + 64, :], t2[:], u[:])

    # ---- phase 3: output projection -------------------------------------
    for b in range(BPC):
        yproj(b)




@functools.cache
def build_nc():
    nc = bacc.Bacc(
        "TRN2",
        target_bir_lowering=False,
        debug=False,
        enable_asserts=False,
        num_devices=NCORES,
    )
    d = {
        "x": nc.dram_tensor("x", [BPC, 2, 128, N], BF16, kind="ExternalInput"),
        "wpack": nc.dram_tensor("wpack", [128, 4096], BF16, kind="ExternalInput"),
        "bpack": nc.dram_tensor("bpack", [128, 10], F32, kind="ExternalInput"),
        "bv": nc.dram_tensor("bv", [1, 512], BF16, kind="ExternalInput"),
        "eb": nc.dram_tensor("eb", [H, 8, 128, N], BF16, kind="ExternalInput"),
        "y": nc.dram_tensor("y", [BPC, 2, 128, N], F32, kind="ExternalOutput"),
    }
    d = {k: (v.ap() if hasattr(v, "ap") else v) for k, v in d.items()}
    with tile.TileContext(nc) as tc:
        with (
            tc.tile_pool(name="main", bufs=1) as pool,
            tc.tile_pool(name="psum", bufs=2, space="PSUM") as psum_pool,
            tc.tile_pool(name="dram", bufs=2, space="DRAM") as dram_pool,
        ):
            d["_pool"] = pool
            d["_psum_pool"] = psum_pool
            d["_dram_pool"] = dram_pool
            build_body(nc, tc, d)
    nc.compile()
    return nc


def _prep_inputs(inputs):
    f = lambda k: np.asarray(inputs[k], np.float32)
    x = f("x")
    sq = f("gq") / np.sqrt(f("vq") + EPS)
    sk = f("gk") / np.sqrt(f("vk") + EPS)
    sv = f("gv") / np.sqrt(f("vv") + EPS)
    so = f("go") / np.sqrt(f("vo") + EPS)
    Wq = f("wq") * sq[:, None] * SCALE
    bq = (f("bq") - f("mq") * sq) * SCALE
    Wk = f("wk") * sk[:, None]
    bk = f("bk") - f("mk") * sk
    Wv = f("wv") * sv[:, None]
    bv = f("bv") - f("mv") * sv
    Wo = f("w_out") * so[:, None] / 6.0
    bo = so * f("b_out") + f("bo") - f("mo") * so

    emb = f("emb")
    pos = np.asarray(inputs["pos_indices"], np.int64)
    EB = np.exp(emb[pos].transpose(2, 0, 1) / SCALE)  # [H, N, N]

    bf = ml_dtypes.bfloat16
    wpack = np.concatenate(
        [
            Wq.T.reshape(2, 128, 512).transpose(1, 0, 2).reshape(128, 1024),
            Wk.T.reshape(2, 128, 512).transpose(1, 0, 2).reshape(128, 1024),
            Wv.T.reshape(2, 128, 512).transpose(1, 0, 2).reshape(128, 1024),
            Wo.T.reshape(4, 128, 256).transpose(1, 0, 2).reshape(128, 1024),
        ],
        axis=1,
    )
    bpack = np.concatenate(
        [bq.reshape(4, 128).T, bk.reshape(4, 128).T, bo.reshape(2, 128).T], axis=1
    )
    shared = {
        "wpack": np.ascontiguousarray(wpack).astype(bf),
        "bpack": np.ascontiguousarray(bpack),
        "bv": bv.reshape(1, 512).astype(bf),
        "eb": np.ascontiguousarray(EB.reshape(H, 8, 128, N)).astype(bf),
    }
    x_dev = x.reshape(B, 2, 128, N).astype(bf)
    in_maps = [
        dict(shared, x=np.ascontiguousarray(x_dev[c * BPC : (c + 1) * BPC]))
        for c in range(NCORES)
    ]
    return in_maps


def kernel(**inputs):
    nc = build_nc()
    in_maps = _prep_inputs(inputs)
    res = run_bass_kernel_spmd(nc, in_maps, core_ids=list(range(NCORES)))
    y = np.concatenate([r["y"].reshape(BPC, DIM, 32, 32) for r in res.results], axis=0)
    return y.astype(np.float32)


def _install_ntff_hook():
    """The image's antenv lacks axon_hooks; synthesize it so trace=True works."""
    import types

    try:
        from antenv.axon_hooks import get_axon_ntff_profile_hook  # noqa: F401

        return
    except ImportError:
        pass
    import antenv
    from trn_agent_boot.trn_boot import _ntff_profile_via_ctypes

    mod = types.ModuleType("antenv.axon_hooks")
    mod._hook = _ntff_profile_via_ctypes("/opt/axon/libaxon_pjrt.so")
    mod.get_axon_ntff_profile_hook = lambda: mod._hook
    mod.set_axon_ntff_profile_hook = lambda h: setattr(mod, "_hook", h)
    sys.modules["antenv.axon_hooks"] = mod
    antenv.axon_hooks = mod

    # no artifact bucket in this container; neuter the upload
    import concourse.bass_utils as bu

    bu.upload_artifacts = lambda tmpdir: f"local:{tmpdir}"


def run_traced(inputs, tmpdir=None):
    """Like kernel() but with NTFF tracing; returns (y, BassKernelResults)."""
    _install_ntff_hook()
    nc = build_nc()
    in_maps = _prep_inputs(inputs)
    res = run_bass_kernel_spmd(
        nc, in_maps, core_ids=list(range(NCORES)), trace=True, tmpdir=tmpdir
    )
    y = np.concatenate([r["y"].reshape(BPC, DIM, 32, 32) for r in res.results], axis=0)
    return y.astype(np.float32), res
